# revision 11
# baseline (speedup 1.0000x reference)
"""Trainium2 Bass kernel for soft decision-tree histogram binning.

Math (per row n of x[N=2048, F=8], cut_points[F, D=3], T=0.1):
    W = [1, 2, 3, 4];  cs = sort(cut_points, axis=1)
    b[f] = cumsum([0, -cs[f,0], -cs[f,1], -cs[f,2]])
    h[n,f,:] = x[n,f] * W + b[f]
    bins[n,f,:] = softmax(h / T)              # [N, F, 4]
    out[n] = kron_f bins[n,f,:]               # [N, 4^8 = 65536]

Strategy: pure data-parallel over 8 NeuronCores (256 rows each). Output is
512 MB fp32 -> HBM-write-bound; the two HWDGE rings sustain ~425 GB/s
(the 16-SDMA-engine fabric ceiling) once both are fed, so everything else
is about (a) time-to-first-byte and (b) never letting the rings starve.

Per 128-row tile: unnormalized exps e[128, 8, 4] (per-feature
max-subtracted, temperature folded into the ACT exp scale), normalized
once via 1/prod(group sums):
  A[16]  = (e0 * 1/P) (x) e1
  B[4096] = e2 (x) e3 (x) e4 (x) e5 (x) e6 (x) e7
  out[:, c*4096+q*1024 : c*4096+(q+1)*1024] = Bq * A[:, c]
B is built in four 1024-col quarters Bq (t2345[q*64:(q+1)*64] (x) t67);
after each quarter, all 16 chunks' matching column-quarters ship as
0.5 MiB tensor_scalar products (0.57 ns/elem -> ~720 GB/s production,
faster than the rings drain, so backlog accumulates and the next
quarter's 1.3 us TT never starves the stream).

Two Tile-scheduler facts this layout is built around (the scheduler pops
each engine's ready list FIFO-by-readiness-time, NOT program order):
  - each Bq+1 build multiplies t67 by a glue scalar == 1.0 exactly
    (firstTS[0:1]*0.0+1.0) whose data dep is quarter q's first output
    chunk, so the scheduler cannot hoist all four Bq TTs ahead of the
    output TSes;
  - tile 1's input broadcast is scaled by the same glue scalar from
    tile 0, so tile 1's small ops can't interleave into tile 0's
    critical path to the first output DMA.
Output DMAs alternate the SP/ACT HWDGE rings per 0.5 MiB piece, with
the roles swapped between the two row tiles: both rings carry exactly
32 MiB and drain together. Keep every DMA's per-partition runs
contiguous: a strided-dest DMA on the ACT ring corrupted HWDGE
semaphore accounting (kernel ended before the stream drained).
"""

import sys

import numpy as np

for _p in ("/opt/trn_rl_repo",):
    if _p not in sys.path:
        sys.path.insert(0, _p)

import concourse.bass as bass
import concourse.tile as tile
from concourse import mybir
from concourse.bass_utils import run_bass_kernel_spmd

TEMPERATURE = 0.1
N, F, NB = 2048, 8, 4  # NB = D+1 bins per feature
NCORES = 8
NLOC = N // NCORES  # 256 rows per core
OUT_COLS = NB**F  # 65536
ROW_TILE = 128
A_COLS = NB * NB  # 16   = kron(e0, e1)
B_COLS = NB**6  # 4096 = kron(e2..e7)
NQ = 4  # B built in NQ quarters of B_COLS/NQ cols each
QCOLS = B_COLS // NQ  # 1024
OQ_BUFS = 12  # 0.5 MiB output staging tiles in flight
f32 = mybir.dt.float32

# test.py can flip these to profile; harness just calls kernel().
RUN_KWARGS: dict = {}
LAST_RESULTS = None

_cache: dict = {}


def _build_nc() -> bass.Bass:
    nc = bass.Bass()
    x_d = nc.declare_dram_parameter("x", [NLOC, F], f32, isOutput=False)
    # consts row layout: [0:4] = W, [4:36] = b[f, j] row-major; replicated x128
    c_d = nc.declare_dram_parameter("consts", [128, NB + F * NB], f32, isOutput=False)
    o_d = nc.declare_dram_parameter("out", [NLOC, OUT_COLS], f32, isOutput=True)

    MUL = mybir.AluOpType.mult
    ADD = mybir.AluOpType.add
    SUB = mybir.AluOpType.subtract
    AX = mybir.AxisListType.X

    with tile.TileContext(nc) as tc:
        with (
            tc.tile_pool(name="singles", bufs=1) as singles,
            tc.tile_pool(name="work", bufs=2) as work,
            tc.tile_pool(name="bq", bufs=2) as bqp,
            tc.tile_pool(name="oq", bufs=OQ_BUFS) as oqp,
        ):
            cst = singles.tile([128, NB + F * NB], f32)
            # contiguous-dest load on the ACT ring: frees the SP ring so the
            # x loads and first output chunks issue earlier
            nc.scalar.dma_start(out=cst, in_=c_d[:])
            # Bounce constants through a DVE copy: TensorTensor's ISA struct
            # has a single sync-wait slot, so TT ops must not wait on a DMA
            # lane and the DVE semaphore at the same time.
            cstS = singles.tile([128, NB + F * NB], f32)
            nc.vector.tensor_copy(cstS[:], cst[:])
            cW = cstS[:, 0:NB]  # [128, 4]
            cB = cstS[:, NB:].rearrange("p (f j) -> p f j", j=NB)  # [128, 8, 4]

            gate = None  # tile 0's glue scalar (== 1.0), gates tile 1's start
            for t in range(NLOC // ROW_TILE):
                r0 = t * ROW_TILE
                # tile 0: even output pieces on SP ring, odd on ACT ring;
                # tile 1: roles swapped -> both rings carry exactly 32 MiB.
                eng_a = nc.sync if t == 0 else nc.scalar
                eng_b = nc.scalar if t == 0 else nc.sync

                xt = work.tile([128, F], f32)
                nc.sync.dma_start(out=xt, in_=x_d[r0 : r0 + ROW_TILE, :])

                # Broadcast-expand x to [128, F, NB] first: keeps every compute
                # instruction waiting on at most ONE DMA semaphore lane (the
                # CoreV3 TensorTensor ISA struct has a single sync-wait slot).
                xe = work.tile([128, F, NB], f32)
                if gate is None:
                    nc.vector.tensor_copy(
                        xe[:], xt.unsqueeze(2).to_broadcast([128, F, NB])
                    )
                else:
                    # x * 1.0 — the data dep on tile 0's glue scalar keeps
                    # this tile's ops out of tile 0's first-DMA window
                    nc.vector.tensor_scalar_mul(
                        xe[:], xt.unsqueeze(2).to_broadcast([128, F, NB]), gate[:, 0:1]
                    )

                # h = x[:, f] * W[j] + b[f, j]
                h = work.tile([128, F, NB], f32)
                nc.vector.tensor_tensor(
                    h[:],
                    xe[:],
                    cW.unsqueeze(1).to_broadcast([128, F, NB]),
                    op=MUL,
                )
                nc.vector.tensor_tensor(h[:], h[:], cB, op=ADD)

                # per-(row, feature) max over the 4 bins, for exp stability
                m = work.tile([128, F], f32)
                nc.vector.reduce_max(m, h[:], axis=AX)
                hm = work.tile([128, F, NB], f32)
                nc.vector.tensor_tensor(
                    hm[:], h[:], m.unsqueeze(2).to_broadcast([128, F, NB]), op=SUB
                )
                # e = exp((h - m) / T)  (scale folds in the temperature)
                e = work.tile([128, F, NB], f32)
                nc.scalar.activation(
                    e[:], hm[:], mybir.ActivationFunctionType.Exp, scale=1.0 / TEMPERATURE
                )

                # group sums -> product over features (one reduce) -> reciprocal
                s = work.tile([128, F], f32)
                nc.vector.reduce_sum(s, e[:], axis=AX)
                p1 = work.tile([128, 1], f32)
                nc.vector.tensor_reduce(p1, s[:], axis=AX, op=MUL)
                rP = work.tile([128, 1], f32)
                nc.vector.reciprocal(rP[:], p1[:])

                # A[16] = (e0 * rP) (x) e1   -- one fused scalar_tensor_tensor
                A16 = work.tile([128, NB, NB], f32)
                nc.vector.scalar_tensor_tensor(
                    A16[:],
                    e[:, 0, :].unsqueeze(2).to_broadcast([128, NB, NB]),
                    rP[:, 0:1],
                    e[:, 1, :].unsqueeze(1).to_broadcast([128, NB, NB]),
                    op0=MUL,
                    op1=MUL,
                )
                A16f = A16.rearrange("p a b -> p (a b)")

                # B[4096] = e2 (x) e3 (x) e4 (x) e5 (x) e6 (x) e7, by quarters
                t23 = work.tile([128, NB, NB], f32)
                nc.vector.tensor_tensor(
                    t23[:],
                    e[:, 2, :].unsqueeze(2).to_broadcast([128, NB, NB]),
                    e[:, 3, :].unsqueeze(1).to_broadcast([128, NB, NB]),
                    op=MUL,
                )
                t45 = work.tile([128, NB, NB], f32)
                nc.vector.tensor_tensor(
                    t45[:],
                    e[:, 4, :].unsqueeze(2).to_broadcast([128, NB, NB]),
                    e[:, 5, :].unsqueeze(1).to_broadcast([128, NB, NB]),
                    op=MUL,
                )
                t67 = work.tile([128, NB, NB], f32)
                nc.vector.tensor_tensor(
                    t67[:],
                    e[:, 6, :].unsqueeze(2).to_broadcast([128, NB, NB]),
                    e[:, 7, :].unsqueeze(1).to_broadcast([128, NB, NB]),
                    op=MUL,
                )
                t23f = t23.rearrange("p a b -> p (a b)")
                t45f = t45.rearrange("p a b -> p (a b)")
                t2345 = work.tile([128, 16, 16], f32)
                nc.vector.tensor_tensor(
                    t2345[:],
                    t23f.unsqueeze(2).to_broadcast([128, 16, 16]),
                    t45f.unsqueeze(1).to_broadcast([128, 16, 16]),
                    op=MUL,
                )
                t2345f = t2345.rearrange("p a b -> p (a b)")

                t67q = t67.rearrange("p a b -> p (a b)")  # current quarter's t67
                ndma = 0
                for q in range(NQ):
                    Bq = bqp.tile([128, 64, 16], f32, tag="bq")
                    nc.vector.tensor_tensor(
                        Bq[:],
                        t2345f[:, q * 64 : (q + 1) * 64]
                        .unsqueeze(2)
                        .to_broadcast([128, 64, 16]),
                        t67q.unsqueeze(1).to_broadcast([128, 64, 16]),
                        op=MUL,
                    )
                    Bqf = Bq.rearrange("p a b -> p (a b)")
                    first_ob = None
                    for c in range(A_COLS):
                        ob = oqp.tile([128, QCOLS], f32, tag="oq")
                        nc.vector.tensor_scalar_mul(ob[:], Bqf, A16f[:, c : c + 1])
                        if first_ob is None:
                            first_ob = ob
                        dma_eng = eng_a if ndma % 2 == 0 else eng_b
                        ndma += 1
                        col0 = c * B_COLS + q * QCOLS
                        dma_eng.dma_start(
                            out=o_d[r0 : r0 + ROW_TILE, col0 : col0 + QCOLS],
                            in_=ob[:],
                        )
                    if q < NQ - 1 or t == 0:
                        # glue scalar == exactly 1.0 (ob[0]*0.0 + 1.0) with a
                        # data dep on this quarter's first output TS: forces
                        # the FIFO-by-readiness scheduler to run the 16 TSes
                        # above before the next 1.3 us Bq TT (and gates the
                        # next row tile's start, via `gate`)
                        g = work.tile([128, 1], f32)
                        nc.vector.tensor_scalar(
                            g[:], first_ob[:, 0:1], 0.0, 1.0, op0=MUL, op1=ADD
                        )
                        if q < NQ - 1:
                            t67g = work.tile([128, 16], f32)
                            nc.vector.tensor_scalar_mul(t67g[:], t67q, g[:, 0:1])
                            t67q = t67g[:]
                        elif t == 0:
                            gate = g
    return nc


def _split_multi_waits(nc: bass.Bass) -> None:
    """Walrus' CoreV3 compute-ISA structs carry a single sync-wait slot, but
    Tile (with optimize_sems disabled) can attach 2+ waits to one compute
    instruction. Hoist all but one wait onto dedicated same-engine NoOps
    inserted right before the instruction — the engine blocks on each in
    program order, so semantics are identical."""
    skip = {"InstEventSemaphore", "InstNoOp"}
    counter = [0]
    for fn in nc.m.functions:
        for bb in fn.blocks:
            insts = bb.instructions
            i = 0
            while i < len(insts):
                ins = insts[i]
                si = getattr(ins, "sync_info", None)
                if (
                    type(ins).__name__ not in skip
                    and si is not None
                    and si.on_wait
                    and len(si.on_wait) > 1
                ):
                    extra, keep = si.on_wait[:-1], si.on_wait[-1:]
                    for w in extra:
                        counter[0] += 1
                        nop = mybir.InstEventSemaphore(
                            name=f"I-waitsplit-{counter[0]}",
                            engine=ins.engine,
                            bass_nofuse=True,
                            sync_info=mybir.SyncInfo(on_wait=[w], on_update=[]),
                            bass_scheduled_tick=ins.bass_scheduled_tick,
                            bass_scheduled_proc=ins.bass_scheduled_proc,
                            bass_scheduled_scope=ins.bass_scheduled_scope,
                            debug=ins.debug,
                        )
                        insts.insert(i, nop)
                        i += 1
                    si.on_wait = keep
                i += 1


def _get_nc() -> bass.Bass:
    if "nc" not in _cache:
        nc = _build_nc()
        _split_multi_waits(nc)
        _cache["nc"] = nc
    return _cache["nc"]


def _host_consts(cut_points: np.ndarray) -> np.ndarray:
    cs = np.sort(np.asarray(cut_points, dtype=np.float32), axis=1)  # [F, D]
    b = np.concatenate([np.zeros((F, 1), np.float32), -cs], axis=1)
    b = np.cumsum(b, axis=1, dtype=np.float32)  # [F, 4]
    W = np.linspace(1.0, float(NB), NB).astype(np.float32)  # [1, 2, 3, 4]
    row = np.concatenate([W, b.reshape(-1)]).astype(np.float32)  # [36]
    return np.ascontiguousarray(np.broadcast_to(row, (128, row.size)))


def kernel(x: np.ndarray, cut_points: np.ndarray) -> np.ndarray:
    global LAST_RESULTS
    x = np.ascontiguousarray(x, dtype=np.float32)
    consts = _host_consts(cut_points)
    nc = _get_nc()
    in_maps = [
        {"x": x[i * NLOC : (i + 1) * NLOC], "consts": consts} for i in range(NCORES)
    ]
    res = run_bass_kernel_spmd(nc, in_maps, list(range(NCORES)), **RUN_KWARGS)
    LAST_RESULTS = res
    return np.concatenate([r["out"] for r in res.results], axis=0)


# revision 12
# speedup vs baseline: 1.0842x; 1.0842x over previous
"""Trainium2 Bass kernel for soft decision-tree histogram binning.

Math (per row n of x[N=2048, F=8], cut_points[F, D=3], T=0.1):
    W = [1, 2, 3, 4];  cs = sort(cut_points, axis=1)
    b[f] = cumsum([0, -cs[f,0], -cs[f,1], -cs[f,2]])
    h[n,f,:] = x[n,f] * W + b[f]
    bins[n,f,:] = softmax(h / T)              # [N, F, 4]
    out[n] = kron_f bins[n,f,:]               # [N, 4^8 = 65536]

Strategy: pure data-parallel over 8 NeuronCores (256 rows each). Output is
512 MB fp32 -> HBM-write-bound; the two HWDGE rings together sustain
~425 GB/s (the 16-SDMA-engine fabric ceiling) once fed, so the kernel is
organized around (a) time-to-first-byte and (b) never starving the rings.

Per 128-row tile, with unnormalized exps e[128, 8, 4] (per-feature
max-subtracted, temperature folded into the ACT exp scale) normalized once
via 1/prod(group sums):
  A[64]   = (e0 * 1/P) (x) e1 (x) e2        # high 3 digits -> 1024-col blocks
  B[1024] = e3 (x) e4 (x) e5 (x) e6 (x) e7  # low 5 digits, contiguous
  out[:, a*1024:(a+1)*1024] = B * A[:, a]
The A64/B1024 split (vs A16/B4096) keeps the whole B build to one 1.2 us
TENSOR_TENSOR *before* the output stream starts — there is no multi-us
B-window mid-stream for the DMA rings to starve through. Every output
byte is then produced by [128, 1024] TENSOR_SCALAR ops (DVE 2x perf mode,
~750 GB/s), 4 consecutive blocks staged into one [128, 4096] SBUF tile so
each dma_start moves 2 MiB with contiguous 16 KB per-partition runs
(4 KB runs measurably cap the stream at ~300 GB/s; 16 KB runs reach the
~425 GB/s fabric ceiling).

Scheduling notes (the Tile scheduler pops each engine's ready list
FIFO-by-readiness-time, NOT program order):
  - the serial DVE chain here is naturally in readiness order, so no
    ordering hacks are needed for tile 0;
  - tile 1's input broadcast is multiplied by a glue scalar == exactly 1.0
    (firstTS[0:1]*0.0 + 1.0) whose data dep is tile 0's first output
    block, so tile 1's small ops cannot interleave into tile 0's critical
    path to the first output DMA.
Output DMAs alternate the SP/ACT HWDGE rings per 2 MiB group, roles
swapped between the two row tiles: both rings carry exactly 32 MiB and
drain together (a single ring tops out at ~230 GB/s, so an imbalance
directly lengthens the tail). Keep every DMA's per-partition runs
contiguous: a strided-dest DMA on the ACT ring corrupted HWDGE semaphore
accounting (kernel ended before the stream drained).
"""

import sys

import numpy as np

for _p in ("/opt/trn_rl_repo",):
    if _p not in sys.path:
        sys.path.insert(0, _p)

import concourse.bass as bass
import concourse.tile as tile
from concourse import mybir
from concourse.bass_utils import run_bass_kernel_spmd

TEMPERATURE = 0.1
N, F, NB = 2048, 8, 4  # NB = D+1 bins per feature
NCORES = 8
NLOC = N // NCORES  # 256 rows per core
OUT_COLS = NB**F  # 65536
ROW_TILE = 128
A_COLS = NB**3  # 64 block scalars = kron(e0, e1, e2)
B_COLS = NB**5  # 1024 = kron(e3..e7)
GROUP = 4  # blocks staged per DMA: 4 * 1024 cols = 16 KB/partition
OQ_BUFS = 6  # 2 MiB staging tiles in flight
f32 = mybir.dt.float32

# test.py can flip these to profile; harness just calls kernel().
RUN_KWARGS: dict = {}
LAST_RESULTS = None

_cache: dict = {}


def _build_nc() -> bass.Bass:
    nc = bass.Bass()
    x_d = nc.declare_dram_parameter("x", [NLOC, F], f32, isOutput=False)
    # consts row layout: [0:4] = W, [4:36] = b[f, j] row-major; replicated x128
    c_d = nc.declare_dram_parameter("consts", [128, NB + F * NB], f32, isOutput=False)
    o_d = nc.declare_dram_parameter("out", [NLOC, OUT_COLS], f32, isOutput=True)

    MUL = mybir.AluOpType.mult
    ADD = mybir.AluOpType.add
    SUB = mybir.AluOpType.subtract
    AX = mybir.AxisListType.X

    with tile.TileContext(nc) as tc:
        with (
            tc.tile_pool(name="singles", bufs=1) as singles,
            tc.tile_pool(name="work", bufs=2) as work,
            tc.tile_pool(name="oq", bufs=OQ_BUFS) as oqp,
        ):
            cst = singles.tile([128, NB + F * NB], f32)
            # contiguous-dest load on the ACT ring: frees the SP ring so the
            # x loads and first output chunks issue earlier
            nc.scalar.dma_start(out=cst, in_=c_d[:])
            # Bounce constants through a DVE copy: TensorTensor's ISA struct
            # has a single sync-wait slot, so TT ops must not wait on a DMA
            # lane and the DVE semaphore at the same time.
            cstS = singles.tile([128, NB + F * NB], f32)
            nc.vector.tensor_copy(cstS[:], cst[:])
            cW = cstS[:, 0:NB]  # [128, 4]
            cB = cstS[:, NB:].rearrange("p (f j) -> p f j", j=NB)  # [128, 8, 4]

            gate = None  # tile 0's glue scalar (== 1.0), gates tile 1's start
            for t in range(NLOC // ROW_TILE):
                r0 = t * ROW_TILE
                # tile 0: even groups on SP ring, odd on ACT ring;
                # tile 1: roles swapped -> both rings carry exactly 32 MiB.
                eng_a = nc.sync if t == 0 else nc.scalar
                eng_b = nc.scalar if t == 0 else nc.sync

                xt = work.tile([128, F], f32)
                nc.sync.dma_start(out=xt, in_=x_d[r0 : r0 + ROW_TILE, :])

                # Broadcast-expand x to [128, F, NB] first: keeps every compute
                # instruction waiting on at most ONE DMA semaphore lane (the
                # CoreV3 TensorTensor ISA struct has a single sync-wait slot).
                xe = work.tile([128, F, NB], f32)
                if gate is None:
                    nc.vector.tensor_copy(
                        xe[:], xt.unsqueeze(2).to_broadcast([128, F, NB])
                    )
                else:
                    # x * 1.0 — the data dep on tile 0's glue scalar keeps
                    # this tile's ops out of tile 0's first-DMA window
                    nc.vector.tensor_scalar_mul(
                        xe[:], xt.unsqueeze(2).to_broadcast([128, F, NB]), gate[:, 0:1]
                    )

                # h = x[:, f] * W[j] + b[f, j]
                h = work.tile([128, F, NB], f32)
                nc.vector.tensor_tensor(
                    h[:],
                    xe[:],
                    cW.unsqueeze(1).to_broadcast([128, F, NB]),
                    op=MUL,
                )
                nc.vector.tensor_tensor(h[:], h[:], cB, op=ADD)

                # per-(row, feature) max over the 4 bins, for exp stability
                m = work.tile([128, F], f32)
                nc.vector.reduce_max(m, h[:], axis=AX)
                hm = work.tile([128, F, NB], f32)
                nc.vector.tensor_tensor(
                    hm[:], h[:], m.unsqueeze(2).to_broadcast([128, F, NB]), op=SUB
                )
                # e = exp((h - m) / T)  (scale folds in the temperature)
                e = work.tile([128, F, NB], f32)
                nc.scalar.activation(
                    e[:], hm[:], mybir.ActivationFunctionType.Exp, scale=1.0 / TEMPERATURE
                )

                # group sums -> product over features (one reduce) -> reciprocal
                s = work.tile([128, F], f32)
                nc.vector.reduce_sum(s, e[:], axis=AX)
                p1 = work.tile([128, 1], f32)
                nc.vector.tensor_reduce(p1, s[:], axis=AX, op=MUL)
                rP = work.tile([128, 1], f32)
                nc.vector.reciprocal(rP[:], p1[:])

                # A[64] = ((e0 (x) e1) * rP) (x) e2
                e01 = work.tile([128, NB, NB], f32)
                nc.vector.tensor_tensor(
                    e01[:],
                    e[:, 0, :].unsqueeze(2).to_broadcast([128, NB, NB]),
                    e[:, 1, :].unsqueeze(1).to_broadcast([128, NB, NB]),
                    op=MUL,
                )
                e01f = e01.rearrange("p a b -> p (a b)")
                A64 = work.tile([128, 16, NB], f32)
                nc.vector.scalar_tensor_tensor(
                    A64[:],
                    e01f.unsqueeze(2).to_broadcast([128, 16, NB]),
                    rP[:, 0:1],
                    e[:, 2, :].unsqueeze(1).to_broadcast([128, 16, NB]),
                    op0=MUL,
                    op1=MUL,
                )
                A64f = A64.rearrange("p a b -> p (a b)")

                # B[1024] = e3 (x) e4 (x) e5 (x) e6 (x) e7
                t34 = work.tile([128, NB, NB], f32)
                nc.vector.tensor_tensor(
                    t34[:],
                    e[:, 3, :].unsqueeze(2).to_broadcast([128, NB, NB]),
                    e[:, 4, :].unsqueeze(1).to_broadcast([128, NB, NB]),
                    op=MUL,
                )
                t56 = work.tile([128, NB, NB], f32)
                nc.vector.tensor_tensor(
                    t56[:],
                    e[:, 5, :].unsqueeze(2).to_broadcast([128, NB, NB]),
                    e[:, 6, :].unsqueeze(1).to_broadcast([128, NB, NB]),
                    op=MUL,
                )
                t34f = t34.rearrange("p a b -> p (a b)")
                t56f = t56.rearrange("p a b -> p (a b)")
                t3456 = work.tile([128, 16, 16], f32)
                nc.vector.tensor_tensor(
                    t3456[:],
                    t34f.unsqueeze(2).to_broadcast([128, 16, 16]),
                    t56f.unsqueeze(1).to_broadcast([128, 16, 16]),
                    op=MUL,
                )
                t3456f = t3456.rearrange("p a b -> p (a b)")
                B1024 = work.tile([128, 256, NB], f32)
                nc.vector.tensor_tensor(
                    B1024[:],
                    t3456f.unsqueeze(2).to_broadcast([128, 256, NB]),
                    e[:, 7, :].unsqueeze(1).to_broadcast([128, 256, NB]),
                    op=MUL,
                )
                B1024f = B1024.rearrange("p a b -> p (a b)")

                # out block a = B * A[:, a]; GROUP consecutive blocks share a
                # [128, 4096] staging tile so each store is one 2 MiB DMA
                for g in range(A_COLS // GROUP):
                    ob = oqp.tile([128, GROUP * B_COLS], f32, tag="oq")
                    for j in range(GROUP):
                        a = g * GROUP + j
                        nc.vector.tensor_scalar_mul(
                            ob[:, j * B_COLS : (j + 1) * B_COLS],
                            B1024f,
                            A64f[:, a : a + 1],
                        )
                        if t == 0 and g == 0 and j == 0 and gate is None:
                            # glue scalar == exactly 1.0 (ob[0]*0.0 + 1.0),
                            # data-dependent on tile 0's first output block
                            gate = singles.tile([128, 1], f32)
                            nc.vector.tensor_scalar(
                                gate[:], ob[:, 0:1], 0.0, 1.0, op0=MUL, op1=ADD
                            )
                    dma_eng = eng_a if g % 2 == 0 else eng_b
                    dma_eng.dma_start(
                        out=o_d[
                            r0 : r0 + ROW_TILE,
                            g * GROUP * B_COLS : (g + 1) * GROUP * B_COLS,
                        ],
                        in_=ob[:],
                    )
    return nc


def _split_multi_waits(nc: bass.Bass) -> None:
    """Walrus' CoreV3 compute-ISA structs carry a single sync-wait slot, but
    Tile (with optimize_sems disabled) can attach 2+ waits to one compute
    instruction. Hoist all but one wait onto dedicated same-engine NoOps
    inserted right before the instruction — the engine blocks on each in
    program order, so semantics are identical."""
    skip = {"InstEventSemaphore", "InstNoOp"}
    counter = [0]
    for fn in nc.m.functions:
        for bb in fn.blocks:
            insts = bb.instructions
            i = 0
            while i < len(insts):
                ins = insts[i]
                si = getattr(ins, "sync_info", None)
                if (
                    type(ins).__name__ not in skip
                    and si is not None
                    and si.on_wait
                    and len(si.on_wait) > 1
                ):
                    extra, keep = si.on_wait[:-1], si.on_wait[-1:]
                    for w in extra:
                        counter[0] += 1
                        nop = mybir.InstEventSemaphore(
                            name=f"I-waitsplit-{counter[0]}",
                            engine=ins.engine,
                            bass_nofuse=True,
                            sync_info=mybir.SyncInfo(on_wait=[w], on_update=[]),
                            bass_scheduled_tick=ins.bass_scheduled_tick,
                            bass_scheduled_proc=ins.bass_scheduled_proc,
                            bass_scheduled_scope=ins.bass_scheduled_scope,
                            debug=ins.debug,
                        )
                        insts.insert(i, nop)
                        i += 1
                    si.on_wait = keep
                i += 1


def _get_nc() -> bass.Bass:
    if "nc" not in _cache:
        nc = _build_nc()
        _split_multi_waits(nc)
        _cache["nc"] = nc
    return _cache["nc"]


def _host_consts(cut_points: np.ndarray) -> np.ndarray:
    cs = np.sort(np.asarray(cut_points, dtype=np.float32), axis=1)  # [F, D]
    b = np.concatenate([np.zeros((F, 1), np.float32), -cs], axis=1)
    b = np.cumsum(b, axis=1, dtype=np.float32)  # [F, 4]
    W = np.linspace(1.0, float(NB), NB).astype(np.float32)  # [1, 2, 3, 4]
    row = np.concatenate([W, b.reshape(-1)]).astype(np.float32)  # [36]
    return np.ascontiguousarray(np.broadcast_to(row, (128, row.size)))


def kernel(x: np.ndarray, cut_points: np.ndarray) -> np.ndarray:
    global LAST_RESULTS
    x = np.ascontiguousarray(x, dtype=np.float32)
    consts = _host_consts(cut_points)
    nc = _get_nc()
    in_maps = [
        {"x": x[i * NLOC : (i + 1) * NLOC], "consts": consts} for i in range(NCORES)
    ]
    res = run_bass_kernel_spmd(nc, in_maps, list(range(NCORES)), **RUN_KWARGS)
    LAST_RESULTS = res
    return np.concatenate([r["out"] for r in res.results], axis=0)
